# revision 15
# baseline (speedup 1.0000x reference)
"""Trainium2 Bass kernel for nn_Attention_51384988730000 (sparse_attention).

Data-parallel over batch: 8 batch elements -> 8 NeuronCores, one element each.
Per-core computation (all matmuls in float32r = fp32 data at full PE rate):

  x [1025,768] --LN--> xn --PE-transpose--> xnT [768,1026] (feature-major,
      token dim padded to 1026 so every matmul free-dim is even, as fp32r
      requires; the pad column is garbage but finite)
  k  = kv_w[:, :768]^T @ xnT           (d-major [768,1026])
  v  = xn @ kv_w[:, 768:]              (token-major, + ones column per head;
      the pad key's ones-entry is 0 so it contributes nothing to attention)
  dw = depthwise 5x5 conv on xnT       (25 diagonal matmuls, PSUM-accumulated)
  q  = pw_w @ dw  (+ cls passthrough)  (d-major [768,1026])
  rotary(q, k): partial, pair-swap via PE permutation matmul + DVE combine
  per head: dotsT = k_h @ q_h^T ; attnT = exp(scale*dotsT) (ACT, no max-sub --
            logits are O(1) for this distribution); avT = v_aug^T @ attnT with
            the ones column producing the softmax denominator.
  normalize via GPSIMD partition-broadcast of 1/den ; out = o @ out_w + b
"""

import numpy as np

import concourse.bass as bass
import concourse.tile as tile
from concourse import bacc, mybir
from concourse import bass_utils

F32R = mybir.dt.float32r
F32 = mybir.dt.float32
AF = mybir.ActivationFunctionType
ALU = mybir.AluOpType

DIM = 768
HEADS = 12
DH = 64
NTOK = 1025
NT2 = 1026          # padded on-chip token dim (even)
NSP = 1024
NCH = 6             # 768 / 128
TCH = 9             # token chunks of 128
KSZ = [128] * 8 + [2]   # key/query padded chunk widths (on-chip)
YSZ = [128] * 8 + [1]   # real token counts (DRAM I/O)
IC2 = [(0, 512), (512, 1024), (1024, 1026)]   # bank-aligned chunks (k, S, AV)
IC3 = [(0, 342), (342, 684), (684, 1026)]     # single-bank dots chunks
SCALE = float(DH) ** -0.5
EPS = 1e-5
NCORES = 8


def _bcast_ap(ap, parts):
    """Partition-broadcast view of a 1D DRAM AP."""
    return bass.AP(tensor=ap.tensor, offset=ap.offset, ap=[[0, parts]] + list(ap.ap))


def _f32(ap):
    return ap.bitcast(F32)


def _build_program():
    nc = bacc.Bacc("TRN2", target_bir_lowering=False, debug=False, num_devices=NCORES)

    dr = {}
    dr["x"] = nc.dram_tensor("x", [NTOK, DIM], F32R, kind="ExternalInput").ap()
    dr["wk"] = nc.dram_tensor("wk", [DIM, DIM], F32R, kind="ExternalInput").ap()
    dr["wv"] = nc.dram_tensor("wv", [DIM, DIM], F32R, kind="ExternalInput").ap()
    dr["wq"] = nc.dram_tensor("wq", [DIM, DIM], F32R, kind="ExternalInput").ap()
    dr["wo"] = nc.dram_tensor("wo", [DIM, DIM], F32R, kind="ExternalInput").ap()
    dr["wdiag"] = nc.dram_tensor("wdiag", [NCH, 25, 128, 128], F32R, kind="ExternalInput").ap()
    dr["cosf"] = nc.dram_tensor("cosf", [128, NT2], F32R, kind="ExternalInput").ap()
    dr["sinf"] = nc.dram_tensor("sinf", [128, NT2], F32R, kind="ExternalInput").ap()
    dr["perm"] = nc.dram_tensor("perm", [128, 128], F32R, kind="ExternalInput").ap()
    dr["ident"] = nc.dram_tensor("ident", [128, 128], F32R, kind="ExternalInput").ap()
    dr["ones12"] = nc.dram_tensor("ones12", [128, HEADS], F32R, kind="ExternalInput").ap()
    dr["ones12z"] = nc.dram_tensor("ones12z", [128, HEADS], F32R, kind="ExternalInput").ap()
    dr["vzero"] = nc.dram_tensor("vzero", [1, HEADS * (DH + 1)], F32R, kind="ExternalInput").ap()
    dr["gvec"] = nc.dram_tensor("gvec", [DIM], F32, kind="ExternalInput").ap()
    dr["bvec"] = nc.dram_tensor("bvec", [DIM], F32, kind="ExternalInput").ap()
    dr["obias"] = nc.dram_tensor("obias", [DIM], F32, kind="ExternalInput").ap()
    dr["y"] = nc.dram_tensor("y", [NTOK, DIM], F32, kind="ExternalOutput").ap()

    with tile.TileContext(nc) as tc:
        _emit(nc, tc, dr)
    nc.compile()
    return nc


def _emit(nc, tc, dr):
    from contextlib import ExitStack

    ctx = ExitStack()
    with ctx:
        singles = ctx.enter_context(tc.tile_pool(name="singles", bufs=1))
        acts = ctx.enter_context(tc.tile_pool(name="acts", bufs=1))

        ident = singles.tile([128, 128], F32R, tag="ident")
        nc.sync.dma_start(out=ident, in_=dr["ident"])
        perm_sb = singles.tile([128, 128], F32R, tag="perm")
        nc.sync.dma_start(out=perm_sb, in_=dr["perm"])
        cosf = singles.tile([128, NT2], F32R, tag="cosf")
        nc.sync.dma_start(out=cosf, in_=dr["cosf"])
        sinf = singles.tile([128, NT2], F32R, tag="sinf")
        nc.sync.dma_start(out=sinf, in_=dr["sinf"])
        g_sb = singles.tile([128, NCH], F32, tag="g")
        nc.sync.dma_start(out=g_sb, in_=dr["gvec"].rearrange("(c p) -> p c", p=128))
        b_sb = singles.tile([128, NCH], F32, tag="b")
        nc.sync.dma_start(out=b_sb, in_=dr["bvec"].rearrange("(c p) -> p c", p=128))
        ob_sb = singles.tile([128, DIM], F32, tag="ob")
        nc.sync.dma_start(out=ob_sb, in_=_bcast_ap(dr["obias"], 128))
        cls_sb = singles.tile([128, NCH], F32R, tag="cls")
        eps_sb = singles.tile([128, 1], F32, tag="eps")
        nc.vector.memset(eps_sb, EPS)

        xnTp_ctx = tc.tile_pool(name="xnTp", bufs=1, side="right")
        xnTp = xnTp_ctx.__enter__()
        xnT = [xnTp.tile([128, NT2], F32R, tag=f"xnT{c}", name=f"xnT{c}") for c in range(NCH)]

        # ============ Phase A: load x, layernorm, transpose ============
        with tc.tile_pool(name="xa", bufs=2) as xpool, \
             tc.tile_pool(name="stats", bufs=4) as spool, \
             tc.tile_pool(name="pst", bufs=4, space="PSUM") as pst:
            for t in range(TCH):
                rows = YSZ[t]
                xt = xpool.tile([128, DIM], F32R, tag="x")
                nc.sync.dma_start(out=xt[:rows], in_=dr["x"][t * 128:t * 128 + rows])
                st = spool.tile([128, 3, 6], F32, tag="st")
                xg = xt.rearrange("p (s f) -> p s f", f=256)
                for sg in range(3):
                    nc.vector.bn_stats(st[:, sg, :], xg[:, sg, :])
                mv = spool.tile([128, 2], F32, tag="mv")
                nc.vector.bn_aggr(mv, st)
                std = spool.tile([128, 1], F32, tag="std")
                nc.scalar.activation(std, mv[:, 1:2], AF.Sqrt, bias=eps_sb)
                rstd = spool.tile([128, 1], F32, tag="rstd")
                nc.vector.reciprocal(rstd, std)
                nmr = spool.tile([128, 1], F32, tag="nmr")
                nc.vector.tensor_scalar(out=nmr, in0=mv[:, 0:1], scalar1=rstd,
                                        scalar2=-1.0, op0=ALU.mult, op1=ALU.mult)
                xc = xpool.tile([128, DIM], F32R, tag="xc")
                nc.vector.tensor_scalar(out=xc, in0=xt, scalar1=rstd, scalar2=nmr,
                                        op0=ALU.mult, op1=ALU.add)
                wid = KSZ[t]
                for c in range(NCH):
                    pt = pst.tile([128, 128], F32R, tag="pt")
                    nc.tensor.transpose(pt, xc[:, c * 128:(c + 1) * 128], ident)
                    dst = xnT[c][:, t * 128:t * 128 + wid]
                    nc.scalar.activation(dst, pt[:, 0:wid], AF.Identity,
                                         bias=b_sb[:, c:c + 1], scale=g_sb[:, c:c + 1])
            for c in range(NCH):
                nc.scalar.copy(cls_sb[:, c:c + 1], xnT[c][:, 0:1])

        # ============ Phase B1: k projection (d-major) + rotary ============
        krot, qrot, v_sb = [], [], []

        def rotary(tin, psS_pool, tmp_pool, out_tile):
            """out = tin*cosf + (P.T @ tin)*sinf ; tin is SBUF [128, NT2]."""
            psS = psS_pool.tile([128, NT2], F32, tag="psS")
            for lo, hi in IC2:
                nc.tensor.matmul(psS[:, lo:hi], perm_sb, tin[:, lo:hi], start=True, stop=True)
            tS = tmp_pool.tile([128, NT2], F32R, tag="tS")
            nc.vector.tensor_mul(tS, psS, sinf)
            tC = tmp_pool.tile([128, NT2], F32R, tag="tC")
            nc.vector.tensor_mul(tC, tin, cosf)
            nc.gpsimd.tensor_add(out_tile, tC, tS)

        with tc.tile_pool(name="wkp", bufs=1) as wkp, \
             tc.tile_pool(name="pskp", bufs=1, space="PSUM") as pskp, \
             tc.tile_pool(name="psSk", bufs=1, space="PSUM") as psSk, \
             tc.tile_pool(name="ktmp", bufs=2) as ktmp:
            wk_sb = []
            for kc in range(NCH):
                w = wkp.tile([128, DIM], F32R, tag=f"wk{kc}", name=f"wk{kc}")
                nc.sync.dma_start(out=w, in_=dr["wk"][kc * 128:(kc + 1) * 128])
                wk_sb.append(w)
            for c in range(NCH):
                ps = pskp.tile([128, NT2], F32, tag="psk")
                for kc in range(NCH):
                    st, sp = kc == 0, kc == NCH - 1
                    lhs = wk_sb[kc][:, c * 128:(c + 1) * 128]
                    for lo, hi in IC2:
                        nc.tensor.matmul(ps[:, lo:hi], lhs, xnT[kc][:, lo:hi],
                                         start=st, stop=sp)
                kT = ktmp.tile([128, NT2], F32R, tag="kT")
                nc.scalar.copy(kT, ps)
                krot.append(acts.tile([128, NT2], F32R, tag=f"krot{c}", name=f"krot{c}"))
                rotary(kT, psSk, ktmp, krot[c])

        # ============ Phase B2: v projection (token-major) ============
        with tc.tile_pool(name="wvp", bufs=1) as wvp, \
             tc.tile_pool(name="psvp", bufs=2, space="PSUM") as psvp:
            wv_sb = []
            for c in range(NCH):
                w = wvp.tile([128, DIM], F32R, tag=f"wv{c}", name=f"wv{c}")
                nc.sync.dma_start(out=w, in_=dr["wv"][c * 128:(c + 1) * 128])
                wv_sb.append(w)
            for t in range(TCH):
                rows = KSZ[t]
                ps = psvp.tile([128, DIM], F32, tag="psv")
                for c in range(NCH):
                    st, sp = c == 0, c == NCH - 1
                    lhs = xnT[c][:, t * 128:t * 128 + rows]
                    nc.tensor.matmul(ps[:rows, 0:512], lhs, wv_sb[c][:, 0:512], start=st, stop=sp)
                    nc.tensor.matmul(ps[:rows, 512:768], lhs, wv_sb[c][:, 512:768], start=st, stop=sp)
                v_sb.append(acts.tile([128, HEADS, DH + 1], F32R, tag=f"v{t}", name=f"v{t}"))
                nc.sync.dma_start(out=v_sb[t][:, :, DH:DH + 1],
                                  in_=dr["ones12"].rearrange("p (h o) -> p h o", o=1))
                nc.scalar.copy(v_sb[t][:rows, :, 0:DH],
                               ps[:rows].rearrange("p (h d) -> p h d", d=DH))
                if t == 8:
                    # the padded garbage key must contribute exactly nothing:
                    # zero its whole v_aug row (v values and ones entry)
                    nc.sync.dma_start(
                        out=v_sb[t][1:2, :, :],
                        in_=dr["vzero"].rearrange("p (h d) -> p h d", d=DH + 1))

        # ============ Phase B3: depthwise conv (diag matmuls) ============
        taps = [(0, 0)] + [(dy, dx) for dy in range(-2, 3) for dx in range(-2, 3)
                           if (dy, dx) != (0, 0)]
        dwp_ctx = tc.tile_pool(name="dwp", bufs=1)
        dwp = dwp_ctx.__enter__()
        with tc.tile_pool(name="diag", bufs=4) as dgp, \
             tc.tile_pool(name="psdp", bufs=2, space="PSUM") as psdp:
            dwT = []
            for c in range(NCH):
                ps = psdp.tile([128, NSP], F32, tag="psd")
                psg = ps.rearrange("p (y x) -> p y x", x=32)
                sp_in = xnT[c][:, 1:1025].rearrange("p (y x) -> p y x", x=32)
                for ti, (dy, dx) in enumerate(taps):
                    tap_idx = (dy + 2) * 5 + (dx + 2)
                    dg = dgp.tile([128, 128], F32R, tag="dg")
                    nc.sync.dma_start(out=dg, in_=dr["wdiag"][c, tap_idx])
                    # fp32r needs even x-counts and 8B-aligned dst offsets;
                    # odd-dx taps get an even-aligned main range + fp32 remainder.
                    if dx == 0:
                        xranges = [(0, 32, False)]
                    elif dx == 1:
                        xranges = [(0, 30, False), (30, 31, True)]
                    elif dx == -1:
                        xranges = [(2, 32, False), (1, 2, True)]
                    elif dx == 2:
                        xranges = [(0, 30, False)]
                    else:
                        xranges = [(2, 32, False)]
                    for (xl, xh, f32mm) in xranges:
                        for hh in range(2):
                            y_lo, y_hi = max(hh * 16, -dy), min(hh * 16 + 16, 32 - dy)
                            if y_hi <= y_lo:
                                continue
                            o_ap = psg[:, y_lo:y_hi, xl:xh]
                            i_ap = sp_in[:, y_lo + dy:y_hi + dy, xl + dx:xh + dx]
                            w_ap = dg
                            if f32mm:
                                i_ap, w_ap = _f32(i_ap), _f32(w_ap)
                            nc.tensor.matmul(
                                o_ap, w_ap, i_ap,
                                start=(ti == 0), stop=(ti == len(taps) - 1),
                                skip_group_check=True)
                dt_ = dwp.tile([128, NSP], F32R, tag=f"dwT{c}", name=f"dwT{c}")
                nc.scalar.copy(dt_, ps)
                dwT.append(dt_)
        xnTp_ctx.__exit__(None, None, None)

        # ============ Phase B4: pointwise conv -> q, + rotary ============
        with tc.tile_pool(name="wqp", bufs=1) as wqp, \
             tc.tile_pool(name="psqp", bufs=2, space="PSUM") as psqp, \
             tc.tile_pool(name="psSq", bufs=1, space="PSUM") as psSq, \
             tc.tile_pool(name="qtmp", bufs=2) as qtmp:
            wq_sb = []
            for c in range(NCH):
                w = wqp.tile([128, DIM], F32R, tag=f"wq{c}", name=f"wq{c}")
                nc.sync.dma_start(out=w, in_=dr["wq"][c * 128:(c + 1) * 128])
                wq_sb.append(w)
            for o in range(NCH):
                ps = psqp.tile([128, NSP], F32, tag="psq")
                for c in range(NCH):
                    st, sp = c == 0, c == NCH - 1
                    lhs = wq_sb[c][:, o * 128:(o + 1) * 128]
                    nc.tensor.matmul(ps[:, 0:512], lhs, dwT[c][:, 0:512], start=st, stop=sp)
                    nc.tensor.matmul(ps[:, 512:1024], lhs, dwT[c][:, 512:1024], start=st, stop=sp)
                qT = qtmp.tile([128, NT2], F32R, tag="qT")
                nc.scalar.copy(qT[:, 0:1], cls_sb[:, o:o + 1])
                nc.scalar.copy(qT[:, 1025:1026], cls_sb[:, o:o + 1])
                nc.scalar.copy(qT[:, 1:1025], ps)
                qrot.append(acts.tile([128, NT2], F32R, tag=f"qrot{o}", name=f"qrot{o}"))
                rotary(qT, psSq, qtmp, qrot[o])
        dwp_ctx.__exit__(None, None, None)

        # ============ Phase C: attention (12 heads) ============
        with tc.tile_pool(name="pdp", bufs=4, space="PSUM") as pdp, \
             tc.tile_pool(name="pavp", bufs=1, space="PSUM") as pavp, \
             tc.tile_pool(name="attnp", bufs=4) as attnp, \
             tc.tile_pool(name="asmp", bufs=4) as asmp:
            oT = [acts.tile([128, NT2], F32R, tag=f"oT{c}", name=f"oT{c}") for c in range(NCH)]
            for h in range(HEADS):
                c2, r0 = h // 2, (h % 2) * 64
                q_h = qrot[c2][r0:r0 + 64, :]
                k_h = krot[c2][r0:r0 + 64, :]
                av = pavp.tile([65, NT2], F32, tag="av")
                for j in range(TCH):
                    jr = KSZ[j]
                    lhs = k_h[:, j * 128:j * 128 + jr]
                    at = attnp.tile([128, NT2], F32R, tag="at")
                    for lo, hi in IC3:
                        pd = pdp.tile([128, 342], F32, tag="pd")
                        nc.tensor.matmul(pd[:jr, 0:hi - lo], lhs, q_h[:, lo:hi],
                                         start=True, stop=True)
                        nc.scalar.activation(at[:jr, lo:hi], pd[:jr, 0:hi - lo],
                                             AF.Exp, scale=SCALE)
                    vv = v_sb[j][:jr, h, :]
                    for lo, hi in IC2:
                        nc.tensor.matmul(av[:, lo:hi], vv, at[:jr, lo:hi],
                                         start=(j == 0), stop=(j == TCH - 1))
                den = asmp.tile([1, NT2], F32, tag="den")
                nc.scalar.copy(den, av[64:65, :])
                rden = asmp.tile([1, NT2], F32, tag="rden")
                nc.vector.reciprocal(rden, den)
                dbc = asmp.tile([64, NT2], F32, tag="dbc")
                nc.gpsimd.partition_broadcast(dbc, rden)
                nc.vector.tensor_mul(oT[c2][r0:r0 + 64, :], av[0:64, :], dbc)

        # ============ Phase D: output projection ============
        with tc.tile_pool(name="wop", bufs=1) as wop, \
             tc.tile_pool(name="psop", bufs=2, space="PSUM") as psop, \
             tc.tile_pool(name="osp", bufs=3) as osp:
            wo_sb = []
            for c in range(NCH):
                w = wop.tile([128, DIM], F32R, tag=f"wo{c}", name=f"wo{c}")
                nc.sync.dma_start(out=w, in_=dr["wo"][c * 128:(c + 1) * 128])
                wo_sb.append(w)
            for t in range(TCH):
                rows = KSZ[t]
                ysz = YSZ[t]
                ps = psop.tile([128, DIM], F32, tag="pso")
                for c in range(NCH):
                    st, sp = c == 0, c == NCH - 1
                    lhs = oT[c][:, t * 128:t * 128 + rows]
                    nc.tensor.matmul(ps[:rows, 0:512], lhs, wo_sb[c][:, 0:512], start=st, stop=sp)
                    nc.tensor.matmul(ps[:rows, 512:768], lhs, wo_sb[c][:, 512:768], start=st, stop=sp)
                ot = osp.tile([128, DIM], F32, tag="ot")
                nc.vector.tensor_add(ot[:rows], ps[:rows], ob_sb[:rows])
                nc.sync.dma_start(out=dr["y"][t * 128:t * 128 + ysz], in_=ot[:ysz])


def _host_prep(inputs):
    """Build the per-core input maps from the full problem inputs."""
    x = np.asarray(inputs["x"], np.float32)
    sin = np.asarray(inputs["sin"], np.float32)
    cos = np.asarray(inputs["cos"], np.float32)
    ln_g = np.asarray(inputs["ln_g"], np.float32)
    ln_b = np.asarray(inputs["ln_b"], np.float32)
    dw_w = np.asarray(inputs["dw_w"], np.float32)
    pw_w = np.asarray(inputs["pw_w"], np.float32)
    kv_w = np.asarray(inputs["kv_w"], np.float32)
    out_w = np.asarray(inputs["out_w"], np.float32)
    out_b = np.asarray(inputs["out_b"], np.float32)

    wk = np.ascontiguousarray(kv_w[:, :DIM])
    wv = np.ascontiguousarray(kv_w[:, DIM:])
    wq = np.ascontiguousarray(pw_w[:, :, 0, 0].T)  # [in, out]
    wd = dw_w.reshape(DIM, 25)
    wdiag = np.zeros((NCH, 25, 128, 128), np.float32)
    idx = np.arange(128)
    for c in range(NCH):
        for t in range(25):
            wdiag[c, t, idx, idx] = wd[c * 128:(c + 1) * 128, t]

    cosf = np.ones((128, NT2), np.float32)
    sinf = np.zeros((128, NT2), np.float32)
    for half in (0, 64):
        for d in range(32):
            cosf[half + d, 1:1025] = cos[:, d]
            sinf[half + d, 1:1025] = -sin[:, d] if d % 2 == 0 else sin[:, d]

    perm = np.zeros((128, 128), np.float32)
    for m in range(128):
        d = m % 64
        sw = (m ^ 1) if d < 32 else m
        perm[sw, m] = 1.0

    ident = np.eye(128, dtype=np.float32)
    ones12 = np.ones((128, HEADS), np.float32)
    ones12z = np.zeros((128, HEADS), np.float32)
    ones12z[0, :] = 1.0
    vzero = np.zeros((1, HEADS * (DH + 1)), np.float32)

    shared = dict(wk=wk, wv=wv, wq=wq, wo=out_w, wdiag=wdiag, cosf=cosf, sinf=sinf,
                  perm=perm, ident=ident, ones12=ones12, ones12z=ones12z, vzero=vzero,
                  gvec=ln_g, bvec=ln_b, obias=out_b)
    in_maps = []
    for c in range(NCORES):
        m = dict(shared)
        m["x"] = np.ascontiguousarray(x[c])
        in_maps.append(m)
    return in_maps


_PROGRAM = None


def kernel(**inputs):
    global _PROGRAM
    if _PROGRAM is None:
        _PROGRAM = _build_program()
    in_maps = _host_prep(inputs)
    res = bass_utils.run_bass_kernel_spmd(_PROGRAM, in_maps, core_ids=list(range(NCORES)))
    return np.stack([res.results[c]["y"] for c in range(NCORES)]).astype(np.float32)
